# revision 6
# baseline (speedup 1.0000x reference)
"""Trainium2 Bass kernel for MHA cross-attention (nn_MHACross).

Sharding: 8 cores = 2 batches x 4 head-groups (2 heads each).
Per core (batch b, head group g): q = x[b] @ Wq[g].T ; k,v = xmel[b] @
Wkv[g].T ; RoPE on q,k (scale folded into host cos/sin tables, bf16);
per head scores^T = k_r @ q_r^T; p = exp(scores) (no max subtraction,
scores are O(6), safe in fp32); unnormalized out2 = v^T @ p on the PE;
Z = ones^T @ p as 4-way column-tiled M=1 matmuls (concurrent streams in
separate PE column groups); ao = out2 * (1/Z) with 1/Z broadcast across
partitions by gpsimd; y_partial = ao @ Wout[:, g].T.  Host sums the 4
partial y per batch.

Schedule: inputs are host-permuted into contiguous 1MB chunks and
streamed across both hardware DMA rings (sync + scalar) in need-order;
q pair0 + first k/v chunks land early so attention on head 0 starts
~20us in.  Remaining projections (q pair1, v chunks, k chunks, k head1)
are emitted as prework interleaved into the attention group pipeline so
the PE never idles.  RoPE half-swap runs as partition-offset DVE
multiplies (keeps the scalar engine free for exp, which gates the main
loop).  y DMAs ride a gpsimd software queue.
"""
import sys
sys.path.insert(0, '/opt/trn_rl_repo')
import numpy as np

DIM = 1024
NHEADS = 8
HD = 128          # head dim
HPC = 2           # heads per core
NG = 4            # head groups (cores per batch)
B, T, S = 2, 2048, 3000
NKT = DIM // 128  # contraction tiles
ROPE_BASE = 10000.0
CW = 512          # chunk width (t and s)
PAIR = 2 * CW     # paired t-chunk width for exp
NTC = T // CW     # 4 t-chunks
NSC = (S + CW - 1) // CW   # 6 s-chunks
SPAD = NSC * CW   # 3072 padded S
N_ST = (S + 127) // 128    # 24 s-tiles of 128
G = 4             # s-tiles per pipeline group
NGRP = N_ST // G  # 6 groups per block
DELAY = 2

# fallback flags
Z_COL4 = True          # 4-way column-tiled Z matmuls
ROPE_SWAP = "vector"   # vector | gpsimd | scalar
ZADD = "vector"        # engine for z strip copy/add
BCAST = "gpsimd"       # gpsimd | dram

_cache = {}


def build_nc():
    from concourse import bacc, mybir
    from concourse.tile import TileContext

    f32 = mybir.dt.float32
    bf16 = mybir.dt.bfloat16
    AF = mybir.ActivationFunctionType

    nc = bacc.Bacc("TRN2", target_bir_lowering=False, debug=False,
                   num_devices=8, num_swdge_queues=2)

    xQ = nc.dram_tensor("xQ", [NTC, 128, NKT, CW], bf16, kind="ExternalInput")
    xmQ = nc.dram_tensor("xmQ", [NSC, 128, NKT, CW], bf16, kind="ExternalInput")
    WqT = nc.dram_tensor("WqT", [128, NKT * HPC * HD], bf16, kind="ExternalInput")
    WkT = nc.dram_tensor("WkT", [128, NKT * HPC * HD], bf16, kind="ExternalInput")
    WvT = nc.dram_tensor("WvT", [128, NKT * HPC * HD], bf16, kind="ExternalInput")
    WoT = nc.dram_tensor("WoT", [HPC * HD, DIM], bf16, kind="ExternalInput")
    cosq = nc.dram_tensor("cosq", [128, T], bf16, kind="ExternalInput")
    sinq = nc.dram_tensor("sinq", [128, T], bf16, kind="ExternalInput")
    cosk = nc.dram_tensor("cosk", [128, S], bf16, kind="ExternalInput")
    sink = nc.dram_tensor("sink", [128, S], bf16, kind="ExternalInput")
    y = nc.dram_tensor("y", [T, DIM], bf16, kind="ExternalOutput")
    if BCAST == "dram":
        zsd = nc.dram_tensor("zs", [HPC * NTC, CW], f32, kind="ExternalOutput")

    s_chunks = [(i * CW, min(CW, S - i * CW)) for i in range(NSC)]

    with TileContext(nc) as tc:
        with tc.tile_pool(name="wpool", bufs=1) as wp:
            # ---- persistent tiles ----
            wq = wp.tile([128, NKT, HPC * HD], bf16)
            wk = wp.tile([128, NKT, HPC * HD], bf16)
            wv = wp.tile([128, NKT, HPC * HD], bf16)
            wo = [wp.tile([128, DIM], bf16, name=f"wo{h}", uniquify=True)
                  for h in range(HPC)]
            ones = wp.tile([128, 1], bf16)
            nc.vector.memset(ones[:], 1.0)

            qT_r = [wp.tile([128, T], bf16, name=f"qT{h}", uniquify=True)
                    for h in range(HPC)]
            kT_r = [wp.tile([128, S], bf16, name=f"kT{h}", uniquify=True)
                    for h in range(HPC)]
            v_sb = wp.tile([128, N_ST, HPC * HD], bf16)

            # persistent cos/sin tables (loaded once, reused across heads)
            csq = [(wp.tile([128, CW], bf16, name=f"cq{c}", uniquify=True),
                    wp.tile([128, CW], bf16, name=f"sq{c}", uniquify=True))
                   for c in range(NTC)]
            csk = [(wp.tile([128, CW], bf16, name=f"ck{c}", uniquify=True),
                    wp.tile([128, CW], bf16, name=f"sk{c}", uniquify=True))
                   for c in range(NSC)]

            xm = [wp.tile([128, NKT, CW], bf16, name=f"xm{c}", uniquify=True)
                  for c in range(NSC)]

            # ---- DMA prologue: need-order across both hw rings ----
            # sync ring: wq, x0, wk, xm0, wv, x2, xm2, xm4, wo
            # scalar ring: cos/sin interleaved with x1, x3, xm1, xm3, xm5
            xq = [None] * NTC

            with tc.tile_pool(name="xqp", bufs=1) as xqp, \
                 tc.tile_pool(name="rtP", bufs=2) as rtp, \
                 tc.tile_pool(name="pP", bufs=12) as pP, \
                 tc.tile_pool(name="aoP", bufs=6) as aoP, \
                 tc.tile_pool(name="zrP", bufs=2) as zrP, \
                 tc.tile_pool(name="zbP", bufs=2) as zbP, \
                 tc.tile_pool(name="yP", bufs=2) as yP, \
                 tc.tile_pool(name="psA", bufs=2, space="PSUM") as psA:

                for c in range(NTC):
                    xq[c] = xqp.tile([128, NKT, CW], bf16, name=f"xq{c}",
                                     uniquify=True, tag="xq", bufs=NTC)

                # sync ring: strict need-order for the PE's in-order queue
                nc.sync.dma_start(out=wq[:], in_=WqT[:].rearrange("p (k n) -> p k n", k=NKT))
                nc.sync.dma_start(out=xq[0][:], in_=xQ[0])
                nc.sync.dma_start(out=wk[:], in_=WkT[:].rearrange("p (k n) -> p k n", k=NKT))
                nc.sync.dma_start(out=xm[0][:], in_=xmQ[0])
                nc.sync.dma_start(out=xm[1][:], in_=xmQ[1])
                nc.sync.dma_start(out=xq[2][:], in_=xQ[2])
                nc.sync.dma_start(out=xm[3][:], in_=xmQ[3])

                def dma_cs(dst_pair, cos_d, sin_d, c0, cw):
                    nc.scalar.dma_start(out=dst_pair[0][:, :cw], in_=cos_d[:, c0:c0 + cw])
                    nc.scalar.dma_start(out=dst_pair[1][:, :cw], in_=sin_d[:, c0:c0 + cw])

                dma_cs(csq[0], cosq, sinq, 0, CW)
                dma_cs(csk[0], cosk, sink, 0, CW)
                nc.scalar.dma_start(out=xq[1][:], in_=xQ[1])
                dma_cs(csq[1], cosq, sinq, CW, CW)
                dma_cs(csk[1], cosk, sink, CW, CW)
                nc.scalar.dma_start(out=wv[:], in_=WvT[:].rearrange("p (k n) -> p k n", k=NKT))
                dma_cs(csk[2], cosk, sink, 2 * CW, CW)
                nc.scalar.dma_start(out=xm[2][:], in_=xmQ[2])
                dma_cs(csq[2], cosq, sinq, 2 * CW, CW)
                dma_cs(csk[3], cosk, sink, 3 * CW, CW)
                nc.scalar.dma_start(out=xq[3][:], in_=xQ[3])
                dma_cs(csq[3], cosq, sinq, 3 * CW, CW)
                dma_cs(csk[4], cosk, sink, 4 * CW, CW)
                nc.scalar.dma_start(out=xm[4][:], in_=xmQ[4])
                dma_cs(csk[5], cosk, sink, 5 * CW, S - 5 * CW)

                # gpsimd software ring: late-need bulk
                nc.gpsimd.dma_start(out=xm[5][:], in_=xmQ[5])
                for h in range(HPC):
                    nc.gpsimd.dma_start(out=wo[h][:], in_=WoT[h * HD:(h + 1) * HD, :])

                # ---- projection + RoPE ----
                def rope_from_ps(ps, cos_sb, sin_sb, out_sl, cw):
                    swp = rtp.tile([128, CW], f32, name="swp", tag="rt", bufs=2)
                    if ROPE_SWAP == "vector":
                        nc.vector.tensor_mul(swp[0:64, :cw], ps[64:128, :cw], sin_sb[0:64, :cw])
                        nc.vector.tensor_mul(swp[64:128, :cw], ps[0:64, :cw], sin_sb[64:128, :cw])
                    else:
                        eng = nc.gpsimd if ROPE_SWAP == "gpsimd" else nc.scalar
                        if ROPE_SWAP == "gpsimd":
                            eng.tensor_copy(swp[0:64, :cw], ps[64:128, :cw])
                            eng.tensor_copy(swp[64:128, :cw], ps[0:64, :cw])
                        else:
                            eng.copy(swp[0:64, :cw], ps[64:128, :cw])
                            eng.copy(swp[64:128, :cw], ps[0:64, :cw])
                        nc.vector.tensor_mul(swp[:, :cw], swp[:, :cw], sin_sb[:, :cw])
                    nc.vector.tensor_mul(out_sl, ps[:, :cw], cos_sb[:, :cw])
                    nc.gpsimd.tensor_add(out_sl, out_sl, swp[:, :cw])

                def proj_rope(h, w_sb, src, cs, c0, cw, out_r):
                    ps = psA.tile([128, PAIR], f32, name="prps", tag="sc", bufs=2)
                    for kt in range(NKT):
                        nc.tensor.matmul(
                            ps[:, :cw],
                            w_sb[:, kt, h * HD:(h + 1) * HD],
                            src[:, kt, :cw],
                            start=(kt == 0), stop=(kt == NKT - 1))
                    rope_from_ps(ps, cs[0], cs[1], out_r[:, c0:c0 + cw], cw)

                def q_chunk(c):
                    for h in range(HPC):
                        proj_rope(h, wq, xq[c], csq[c], c * CW, CW, qT_r[h])

                def k_chunk(h, c):
                    c0, cw = s_chunks[c]
                    proj_rope(h, wk, xm[c], csk[c], c0, cw, kT_r[h])

                def v_chunk(c):
                    c0, cw = s_chunks[c]
                    for j in range(G):
                        st = G * c + j
                        s0 = st * 128
                        scnt = min(128, S - s0)
                        vps = psA.tile([128, HPC * HD], f32, name="vps", tag="sc", bufs=2)
                        for kt in range(NKT):
                            nc.tensor.matmul(
                                vps[:scnt, :],
                                xm[c][:, kt, j * 128:j * 128 + scnt],
                                wv[:, kt, :],
                                start=(kt == 0), stop=(kt == NKT - 1))
                        nc.vector.tensor_copy(v_sb[:scnt, st, :], vps[:scnt, :])

                # ---- attention pipeline ----
                # blocks in order: (h0,p0), (h0,p1), (h1,p0), (h1,p1)
                BLOCKS = [(0, 0), (0, 1), (1, 0), (1, 1)]
                blocks = {}

                def sc_exp(bi, g):
                    h, pi = BLOCKS[bi]
                    bk = blocks.setdefault(bi, {"pt": {}})
                    for j in range(G):
                        st = G * g + j
                        s0 = st * 128
                        scnt = min(128, S - s0)
                        scps = psA.tile([128, PAIR], f32, name="scps", tag="sc", bufs=2)
                        for ci in range(2):
                            c0 = pi * PAIR + ci * CW
                            nc.tensor.matmul(
                                scps[:scnt, ci * CW:(ci + 1) * CW],
                                kT_r[h][:, s0:s0 + scnt],
                                qT_r[h][:, c0:c0 + CW],
                                start=True, stop=True,
                                skip_group_check=True)
                        p_t = pP.tile([128, PAIR], bf16, name="p_t", tag="p", bufs=12)
                        nc.scalar.activation(p_t[:scnt, :], scps[:scnt, :], AF.Exp)
                        bk["pt"][st] = (p_t, scnt)

                def zav(bi, g):
                    h, pi = BLOCKS[bi]
                    bk = blocks[bi]
                    last = (g == NGRP - 1)
                    if g == 0:
                        bk["zps"] = psA.tile([128, CW], f32, name="zps", tag="z", bufs=2)
                        bk["o2"] = [psA.tile([128, CW], f32, name="o2ps", tag="acc", bufs=2)
                                    for _ in range(2)]
                    zps = bk["zps"]
                    # Z: 4 concurrent M=1 streams in distinct column groups
                    for j in range(G):
                        st = G * g + j
                        p_t, scnt = bk["pt"][st]
                        for ci in range(2):
                            if Z_COL4:
                                strip = 64 * ci + 32 * (st % 2)
                                nc.tensor.matmul(
                                    zps[strip:strip + 1, :CW],
                                    ones[:scnt, :1],
                                    p_t[:scnt, ci * CW:(ci + 1) * CW],
                                    start=(st < 2), stop=(st >= N_ST - 2),
                                    tile_position=(0, strip),
                                    skip_group_check=True)
                            else:
                                strip = 64 * ci
                                nc.tensor.matmul(
                                    zps[strip:strip + 1, :CW],
                                    ones[:scnt, :1],
                                    p_t[:scnt, ci * CW:(ci + 1) * CW],
                                    start=(st == 0), stop=(st == N_ST - 1),
                                    tile_position=(0, strip),
                                    skip_group_check=True)
                    if last:
                        bk["zr2"] = []
                        for ci in range(2):
                            zr = zrP.tile([1, CW], f32, name="zr", tag="zr", bufs=2)
                            if Z_COL4:
                                if ZADD == "vector":
                                    nc.vector.tensor_copy(zr[0:1, :], zps[64 * ci:64 * ci + 1, :])
                                    nc.vector.tensor_add(zr[0:1, :], zr[0:1, :],
                                                         zps[64 * ci + 32:64 * ci + 33, :])
                                else:
                                    nc.scalar.copy(zr[0:1, :], zps[64 * ci:64 * ci + 1, :])
                                    nc.vector.tensor_add(zr[0:1, :], zr[0:1, :],
                                                         zps[64 * ci + 32:64 * ci + 33, :])
                            else:
                                nc.vector.tensor_copy(zr[0:1, :], zps[64 * ci:64 * ci + 1, :])
                            zrec = zrP.tile([1, CW], f32, name="zrec", tag="zc", bufs=2)
                            nc.vector.reciprocal_approx_fast(out=zrec[0:1, :], in_=zr[0:1, :])
                            zr2 = zbP.tile([128, CW], f32, name="zr2", tag="zb", bufs=2)
                            if BCAST == "gpsimd":
                                nc.gpsimd.partition_broadcast(zr2[:, :], zrec[0:1, :])
                            else:
                                zrow = h * NTC + pi * 2 + ci
                                nc.sync.dma_start(out=zsd[zrow:zrow + 1, :], in_=zrec[0:1, :])
                                nc.sync.dma_start(out=zr2[:, :],
                                                  in_=zsd[zrow, :].partition_broadcast(128))
                            bk["zr2"].append(zr2)
                    # attn @ V
                    for j in range(G):
                        st = G * g + j
                        p_t, scnt = bk["pt"][st]
                        for ci in range(2):
                            nc.tensor.matmul(
                                bk["o2"][ci][:, :CW],
                                v_sb[:scnt, st, h * HD:(h + 1) * HD],
                                p_t[:scnt, ci * CW:(ci + 1) * CW],
                                start=(st == 0), stop=(st == N_ST - 1),
                                skip_group_check=True)
                    if last:
                        bk["ao"] = []
                        for ci in range(2):
                            ao_t = aoP.tile([128, CW], bf16, name="ao", tag="ao", bufs=6)
                            nc.vector.tensor_mul(ao_t[:, :], bk["o2"][ci][:, :], bk["zr2"][ci][:, :])
                            bk["ao"].append(ao_t)

                def outproj(pi):
                    b0 = blocks[BLOCKS.index((0, pi))]
                    b1 = blocks[BLOCKS.index((1, pi))]
                    for tt in range(PAIR // 128):
                        ci, tl = tt // 4, (tt % 4) * 128
                        y_sb = yP.tile([128, DIM], bf16, name="y_sb", tag="ysb", bufs=2)
                        for nn in range(2):
                            yps = psA.tile([128, CW], f32, name="yps", tag="sc", bufs=2)
                            for hh, bk in enumerate((b0, b1)):
                                nc.tensor.matmul(
                                    yps[:, :],
                                    bk["ao"][ci][:, tl:tl + 128],
                                    wo[hh][:, nn * CW:(nn + 1) * CW],
                                    start=(hh == 0), stop=(hh == 1),
                                    skip_group_check=True)
                            if pi == 1:
                                nc.scalar.copy(y_sb[:, nn * CW:(nn + 1) * CW], yps[:, :])
                            else:
                                nc.vector.tensor_copy(y_sb[:, nn * CW:(nn + 1) * CW], yps[:, :])
                        r0 = pi * PAIR + tt * 128
                        nc.sync.dma_start(out=y[r0:r0 + 128, :], in_=y_sb[:, :])

                # prologue: q pair0, first k chunk
                q_chunk(0)
                q_chunk(1)
                k_chunk(0, 0)

                # prework interleaved into the pipeline, keyed by (bi, g)
                prework = {}
                for g in range(NGRP):
                    w = []
                    if g == 2:
                        w.append(lambda: q_chunk(2))
                    if g == 3:
                        w.append(lambda: q_chunk(3))
                    w.append(lambda c=g: v_chunk(c))
                    if g + 1 < NSC:
                        w.append(lambda c=g + 1: k_chunk(0, c))
                    prework[(0, g)] = w
                for g in range(NGRP):
                    prework[(1, g)] = [lambda c=g: k_chunk(1, c)]

                groups = [(bi, g) for bi in range(4) for g in range(NGRP)]

                def finish(key):
                    bi, g = key
                    zav(bi, g)
                    if g == NGRP - 1:
                        h, pi = BLOCKS[bi]
                        if h == 1:
                            outproj(pi)

                for i, key in enumerate(groups):
                    for w in prework.get(key, ()):
                        w()
                    sc_exp(*key)
                    if i >= DELAY:
                        finish(groups[i - DELAY])
                for j in range(max(0, len(groups) - DELAY), len(groups)):
                    finish(groups[j])

    nc.compile()
    return nc


def _host_tables():
    scale = float(HD) ** (-0.25)
    inv = 1.0 / (ROPE_BASE ** (np.arange(0, HD, 2, dtype=np.float64) / HD))  # [64]

    def tables(L):
        fr = np.outer(inv, np.arange(L, dtype=np.float64))  # [64, L]
        c = np.cos(fr) * scale
        s = np.sin(fr) * scale
        cos = np.concatenate([c, c], axis=0)
        sin = np.concatenate([-s, s], axis=0)
        return cos, sin

    return tables(T), tables(S)


def make_in_maps(x, xmel, Wq, Wkv, Wout):
    import ml_dtypes
    bf = ml_dtypes.bfloat16
    (cosq_, sinq_), (cosk_, sink_) = _host_tables()
    cosq_, sinq_ = cosq_.astype(bf), sinq_.astype(bf)
    cosk_, sink_ = cosk_.astype(bf), sink_.astype(bf)

    x = np.asarray(x, dtype=np.float32)
    xmel = np.asarray(xmel, dtype=np.float32)
    Wq = np.asarray(Wq, dtype=np.float32)
    Wkv = np.asarray(Wkv, dtype=np.float32)
    Wout = np.asarray(Wout, dtype=np.float32)

    # x[b]: [T, DIM] -> [NTC, 128, NKT, CW] with xQ[c,p,k,t] = x[c*CW+t, k*128+p]
    xQ_b = [np.ascontiguousarray(
        x[b].reshape(NTC, CW, NKT, 128).transpose(0, 3, 2, 1)).astype(bf)
        for b in range(B)]
    xmp = np.zeros((B, SPAD, DIM), dtype=np.float32)
    xmp[:, :S, :] = xmel
    xmQ_b = [np.ascontiguousarray(
        xmp[b].reshape(NSC, CW, NKT, 128).transpose(0, 3, 2, 1)).astype(bf)
        for b in range(B)]

    gsz = HPC * HD  # 256
    WqT_g, WkT_g, WvT_g, WoT_g = [], [], [], []
    for g in range(NG):
        r0 = g * gsz

        def prearr(wt):  # [DIM, gsz] -> [128, NKT*gsz], row p holds [kt, n]
            return np.ascontiguousarray(
                wt.reshape(NKT, 128, gsz).transpose(1, 0, 2).reshape(128, NKT * gsz)).astype(bf)

        WqT_g.append(prearr(Wq[r0:r0 + gsz, :].T))
        WkT_g.append(prearr(Wkv[r0:r0 + gsz, :].T))
        WvT_g.append(prearr(Wkv[DIM + r0:DIM + r0 + gsz, :].T))
        WoT_g.append(np.ascontiguousarray(Wout[:, r0:r0 + gsz].T).astype(bf))

    in_maps = []
    for c in range(B * NG):
        b, g = c // NG, c % NG
        in_maps.append({
            "xQ": xQ_b[b], "xmQ": xmQ_b[b],
            "WqT": WqT_g[g], "WkT": WkT_g[g], "WvT": WvT_g[g], "WoT": WoT_g[g],
            "cosq": cosq_, "sinq": sinq_, "cosk": cosk_, "sink": sink_,
        })
    return in_maps


def kernel(x, xmel, Wq, Wkv, Wout):
    from concourse.bass_utils import run_bass_kernel_spmd

    x = np.asarray(x, dtype=np.float32)
    xmel = np.asarray(xmel, dtype=np.float32)
    Bx, Tx, C = x.shape
    Sx = xmel.shape[1]
    assert (Bx, Tx, C, Sx) == (B, T, DIM, S)

    if "nc" not in _cache:
        _cache["nc"] = build_nc()
    nc = _cache["nc"]

    in_maps = make_in_maps(x, xmel,
                           np.asarray(Wq, dtype=np.float32),
                           np.asarray(Wkv, dtype=np.float32),
                           np.asarray(Wout, dtype=np.float32))
    res = run_bass_kernel_spmd(nc, in_maps, list(range(8)))
    out = np.zeros((B, T, DIM), dtype=np.float32)
    for c in range(8):
        b = c // NG
        out[b] += res.results[c]["y"].astype(np.float32)
    return out
